# revision 10
# baseline (speedup 1.0000x reference)
"""GroupedQueryAttention (B=2,S=2048,D=2048,H=32,KV=8,HD=64) on 8 trn2 cores.

Sharding: data-parallel over batch (2) x tensor-parallel over KV-head quarters
(4): core c = 4*b + j handles batch b, KV groups {2j, 2j+1} (8 Q heads).
Each core computes a partial o_proj output [S, D]; the host sums the 4
partials per batch (the "all-reduce after o_proj" done host-side at gather).

On-device layout notes:
- All matmuls run in float32r (single-pass fp32, ~1.6e-4 rel err, full PE
  rate at N>=256). The BIR verifier requires every f32r matmul operand to be
  *produced* by an f32r-rounding instruction, hence the DVE rounding copies
  after DMA loads.
- Attention works in transposed layouts: QT/KT are [head_dim, S] so scores
  come out as [k, q] tiles, exp runs on ScalarE straight out of PSUM, and
  P^T tiles feed PV matmuls directly as rhs. The softmax denominator is a
  ones-weights matmul accumulated alongside PV; both heads of a pair are
  packed into one 128-partition tile (row-split for scores, col-split via
  tile_position for PV/denom) so normalization is a single aligned DVE mult.
- RoPE is applied in the transposed layout with a block-diagonal signed
  permutation matmul (rotate_half) + cos/sin elementwise combines.
"""
import sys
sys.path.insert(0, "/opt/trn_rl_repo")

import numpy as np

import concourse.bass as bass
import concourse.tile as tile
from concourse import bacc, mybir
from concourse.bass_utils import run_bass_kernel_spmd

P = 128
B, S, D = 2, 2048, 2048
H, KV, HD = 32, 8, 64
NREP = H // KV
N_CORES = 8

HEADS_PER_CORE = 8          # 2 KV groups x 4 reps
QCOLS = HEADS_PER_CORE * HD  # 512
KVCOLS = 2 * HD              # 128
SC = 256                     # phase-A s-chunk width
NSC = S // SC                # 8
DO = D // P                  # 16 contraction chunks
QC = 512                     # phase-B q-chunk width
NQC = S // QC                # 4
NKC = S // P                 # 16 k chunks of 128
# local head order inside the 512 projection columns: tile mp holds heads
# (mp, mp+4) stacked 64+64 on partitions
HEAD_ORDER = [0, 4, 1, 5, 2, 6, 3, 7]

F32 = mybir.dt.float32
F32R = mybir.dt.float32r
EXP = mybir.ActivationFunctionType.Exp

_CACHE = {}


def _rot_matrix() -> np.ndarray:
    """rot[d_in, d_out] s.t. (rot.T @ qT) = rotate_half(q).T per 64-block."""
    blk = np.zeros((HD, HD), dtype=np.float32)
    half = HD // 2
    for d in range(half):
        blk[d + half, d] = -1.0
    for d in range(half, HD):
        blk[d - half, d] = 1.0
    rot = np.zeros((P, P), dtype=np.float32)
    rot[:HD, :HD] = blk
    rot[HD:, HD:] = blk
    return rot


def _build(causal: bool):
    nc = bacc.Bacc("TRN2", target_bir_lowering=False, debug=False,
                   num_devices=N_CORES)
    xT = nc.dram_tensor("xT", [D, S], F32, kind="ExternalInput").ap()
    wq = nc.dram_tensor("wq", [D, QCOLS], F32, kind="ExternalInput").ap()
    wk = nc.dram_tensor("wk", [D, KVCOLS], F32, kind="ExternalInput").ap()
    wv = nc.dram_tensor("wv", [D, KVCOLS], F32, kind="ExternalInput").ap()
    wo = nc.dram_tensor("wo", [QCOLS, D], F32, kind="ExternalInput").ap()
    cs = nc.dram_tensor("cs", [P, S], F32, kind="ExternalInput").ap()
    sn = nc.dram_tensor("sn", [P, S], F32, kind="ExternalInput").ap()
    mk = nc.dram_tensor("mk", [P, 4, QC], F32, kind="ExternalInput").ap()
    rt = nc.dram_tensor("rt", [P, P], F32, kind="ExternalInput").ap()
    y = nc.dram_tensor("y", [S, D], F32, kind="ExternalOutput").ap()

    xT_t = xT.rearrange("(o p) s -> p o s", p=P)
    wq_t = wq.rearrange("(o p) c -> p o c", p=P)
    wk_t = wk.rearrange("(o p) c -> p o c", p=P)
    wv_t = wv.rearrange("(o p) c -> p o c", p=P)
    wo_t = wo.rearrange("(j p) c -> p j c", p=P)

    with tile.TileContext(nc) as tc:
        # tensors that live for the whole kernel
        with tc.tile_pool(name="persist", bufs=1) as pp:
            qt_all = pp.tile([P, 4, S], F32R)     # per pair-tile mp: heads (mp, mp+4)
            kt_all = pp.tile([P, S], F32R)        # g0 on rows 0:64, g1 on 64:128
            # PV lhsT operands, zero-padded so head A lands on out rows 0:64
            # and head B on 64:128 (col-tiling is invalid ISA for f32r; instead
            # both matmuls write full M=128 and accumulate zeros elsewhere)
            v_all = pp.tile([P, NKC, 2, P], F32R)  # slot0 [Vg0|0], slot1 [0|Vg1]
            ones_a = pp.tile([P, P], F32R)         # [1|0] -> denA rows 0:64
            ones_b = pp.tile([P, P], F32R)         # [0|1] -> denB rows 64:128

            # ---- Phase A: projections + RoPE ----
            with tc.tile_pool(name="wA", bufs=1) as wp, \
                 tc.tile_pool(name="stageA", bufs=1) as stp, \
                 tc.tile_pool(name="xstA", bufs=1) as xstp, \
                 tc.tile_pool(name="xA", bufs=2) as xp, \
                 tc.tile_pool(name="ropeA", bufs=2) as rp, \
                 tc.tile_pool(name="psA", bufs=2, space="PSUM") as psa:
                wq_r = wp.tile([P, DO, QCOLS], F32R)
                wk_r = wp.tile([P, DO, KVCOLS], F32R)
                wv_r = wp.tile([P, DO, KVCOLS], F32R)
                rot_r = wp.tile([P, P], F32R)
                # stage + round weights in 8KB/partition slabs
                for (dst, src, cols) in ((wq_r, wq_t, QCOLS),
                                         (wk_r, wk_t, KVCOLS),
                                         (wv_r, wv_t, KVCOLS)):
                    for ob in range(0, DO, 4):
                        st = stp.tile([P, 4, QCOLS], F32, tag="wstage")
                        view = st[:, :, :cols]
                        nc.sync.dma_start(view, src[:, ob:ob + 4, :])
                        nc.vector.tensor_copy(dst[:, ob:ob + 4, :], view)
                st = stp.tile([P, 4, QCOLS], F32, tag="wstage")
                nc.sync.dma_start(st[:, 0, :P], rt)
                nc.vector.tensor_copy(rot_r[:], st[:, 0, :P])
                # constant fills (memset straight into f32r is invalid ISA:
                # round through an f32 staging tile instead)
                fz = stp.tile([P, NKC, HD], F32, tag="fill")
                nc.vector.memset(fz[:], 0.0)
                nc.vector.tensor_copy(v_all[:, :, 0, HD:P], fz[:])
                nc.vector.tensor_copy(v_all[:, :, 1, 0:HD], fz[:])
                nc.vector.tensor_copy(ones_a[:, HD:P], fz[:, 0, :])
                nc.vector.tensor_copy(ones_b[:, 0:HD], fz[:, 0, :])
                fo = stp.tile([P, NKC, HD], F32, tag="fill")
                nc.vector.memset(fo[:, 0, :], 1.0)
                nc.vector.tensor_copy(ones_a[:, 0:HD], fo[:, 0, :])
                nc.vector.tensor_copy(ones_b[:, HD:P], fo[:, 0, :])

                def rope_combine(out_ap, psum_proj, cs_t, sn_t, width):
                    """out = proj*cos + (rot.T @ proj)*sin, all [P, width]."""
                    sb = rp.tile([P, SC], F32R, tag="ropesb")
                    nc.vector.tensor_copy(sb[:, :width], psum_proj)
                    psr = psa.tile([P, SC], F32, tag="rot")
                    nc.tensor.matmul(psr[:, :width], rot_r[:], sb[:, :width],
                                     start=True, stop=True)
                    t1 = rp.tile([P, SC], F32, tag="t1")
                    nc.vector.tensor_mul(t1[:, :width], psr[:, :width],
                                         sn_t[:, :width])
                    t2 = rp.tile([P, SC], F32, tag="t2")
                    nc.vector.tensor_mul(t2[:, :width],
                                         sb.bitcast(F32)[:, :width],
                                         cs_t[:, :width])
                    nc.vector.tensor_add(out_ap, t1[:, :width], t2[:, :width])

                for sc in range(NSC):
                    s0 = sc * SC
                    xst = xstp.tile([P, DO, SC], F32, tag="xstage")
                    nc.sync.dma_start(xst[:], xT_t[:, :, s0:s0 + SC])
                    xr = xp.tile([P, DO, SC], F32R, tag="xr")
                    nc.vector.tensor_copy(xr[:], xst[:])
                    cs_t = rp.tile([P, SC], F32, tag="cs")
                    sn_t = rp.tile([P, SC], F32, tag="sn")
                    nc.sync.dma_start(cs_t[:], cs[:, s0:s0 + SC])
                    nc.sync.dma_start(sn_t[:], sn[:, s0:s0 + SC])
                    for mp in range(4):
                        psq = psa.tile([P, SC], F32, tag="proj")
                        for o in range(DO):
                            nc.tensor.matmul(
                                psq[:], wq_r[:, o, mp * P:(mp + 1) * P],
                                xr[:, o], start=(o == 0), stop=(o == DO - 1))
                        rope_combine(qt_all[:, mp, s0:s0 + SC], psq[:],
                                     cs_t, sn_t, SC)
                    psk = psa.tile([P, SC], F32, tag="proj")
                    for o in range(DO):
                        nc.tensor.matmul(psk[:], wk_r[:, o], xr[:, o],
                                         start=(o == 0), stop=(o == DO - 1))
                    rope_combine(kt_all[:, s0:s0 + SC], psk[:], cs_t, sn_t, SC)
                    for st_i in range(SC // P):
                        t = sc * (SC // P) + st_i
                        psv = psa.tile([P, P], F32, tag="v")
                        for o in range(DO):
                            nc.tensor.matmul(
                                psv[:], xr[:, o, st_i * P:(st_i + 1) * P],
                                wv_r[:, o], start=(o == 0), stop=(o == DO - 1))
                        nc.vector.tensor_copy(v_all[:, t, 0, 0:HD],
                                              psv[:, 0:HD])
                        nc.vector.tensor_copy(v_all[:, t, 1, HD:P],
                                              psv[:, HD:P])

            # ---- Phase B: attention ----
            # ot_all lives in its own pool (phases B+C) so its 32KB/partition
            # don't count against phase A's budget
            ot_pool = tc.alloc_tile_pool(name="otBC", bufs=1)
            ot_all = ot_pool.tile([P, 4, S], F32R)
            with tc.tile_pool(name="mB", bufs=1) as mbp, \
                 tc.tile_pool(name="ptB", bufs=3) as ptp, \
                 tc.tile_pool(name="rbB", bufs=2) as rbp, \
                 tc.tile_pool(name="psSB", bufs=3, space="PSUM") as pss, \
                 tc.tile_pool(name="psVB", bufs=1, space="PSUM") as psv_pool:
                mk_sb = mbp.tile([P, 4, QC], F32)
                nc.sync.dma_start(mk_sb[:], mk)
                for mp in range(4):
                    for qc in range(NQC):
                        q0 = qc * QC
                        nkc = (4 * qc + 4) if causal else NKC
                        pv = psv_pool.tile([P, QC], F32, tag="pv")
                        dn = psv_pool.tile([P, QC], F32, tag="dn")
                        for kc in range(nkc):
                            k0 = kc * P
                            ssl = pss.tile([P, 2, QC], F32, tag="s")
                            nc.tensor.matmul(
                                ssl[:, 0], kt_all[0:HD, k0:k0 + P],
                                qt_all[0:HD, mp, q0:q0 + QC],
                                start=True, stop=True, tile_position=(0, 0))
                            nc.tensor.matmul(
                                ssl[:, 1], kt_all[HD:P, k0:k0 + P],
                                qt_all[HD:P, mp, q0:q0 + QC],
                                start=True, stop=True, tile_position=(HD, 0))
                            pt = ptp.tile([P, 2, QC], F32R, tag="pt")
                            nc.scalar.activation(pt[:], ssl[:], EXP, scale=0.125)
                            if causal and kc >= 4 * qc:
                                dg = kc - 4 * qc
                                nc.vector.tensor_mul(pt[:, 0], pt[:, 0],
                                                     mk_sb[:, dg])
                                nc.vector.tensor_mul(pt[:, 1], pt[:, 1],
                                                     mk_sb[:, dg])
                            first, last = (kc == 0), (kc == nkc - 1)
                            nc.tensor.matmul(pv[:], v_all[:, kc, 0], pt[:, 0],
                                             start=first, stop=False)
                            nc.tensor.matmul(pv[:], v_all[:, kc, 1], pt[:, 1],
                                             start=False, stop=last)
                            nc.tensor.matmul(dn[:], ones_a[:], pt[:, 0],
                                             start=first, stop=False)
                            nc.tensor.matmul(dn[:], ones_b[:], pt[:, 1],
                                             start=False, stop=last)
                        rb = rbp.tile([P, QC], F32, tag="rb")
                        nc.vector.reciprocal_approx_fast(rb[:], dn[:])
                        nc.vector.tensor_mul(ot_all[:, mp, q0:q0 + QC],
                                             pv[:], rb[:])

            # ---- Phase C: output projection (partial; host sums shards) ----
            with tc.tile_pool(name="woC", bufs=2) as wcp, \
                 tc.tile_pool(name="yC", bufs=3) as ycp, \
                 tc.tile_pool(name="psC", bufs=2, space="PSUM") as psc:
                for ncol in range(4):
                    c0 = ncol * 512
                    wst = wcp.tile([P, 4, 512], F32, tag="wostage")
                    nc.sync.dma_start(wst[:], wo_t[:, :, c0:c0 + 512])
                    wo_r = wcp.tile([P, 4, 512], F32R, tag="wor")
                    nc.vector.tensor_copy(wo_r[:], wst[:])
                    for qt in range(S // P):
                        r0 = qt * P
                        psy = psc.tile([P, 512], F32, tag="y")
                        for j in range(4):
                            nc.tensor.matmul(psy[:], ot_all[:, j, r0:r0 + P],
                                             wo_r[:, j],
                                             start=(j == 0), stop=(j == 3))
                        ysb = ycp.tile([P, 512], F32, tag="ysb")
                        nc.vector.tensor_copy(ysb[:], psy[:])
                        nc.sync.dma_start(y[r0:r0 + P, c0:c0 + 512], ysb[:])
            ot_pool.release()
    nc.compile()
    return nc


def _get_nc(causal: bool):
    if causal not in _CACHE:
        _CACHE[causal] = _build(causal)
    return _CACHE[causal]


def _make_in_maps(x, cos, sin, Wq, Wk, Wv, Wo):
    cos_t = np.ascontiguousarray(cos.reshape(S, HD).T).astype(np.float32)
    sin_t = np.ascontiguousarray(sin.reshape(S, HD).T).astype(np.float32)
    cs = np.vstack([cos_t, cos_t])
    sn = np.vstack([sin_t, sin_t])
    kk = np.arange(P)[:, None]
    qq = np.arange(QC)[None, :]
    mk = np.stack([(kk + P * d <= qq) for d in range(4)], axis=1).astype(
        np.float32)  # [P, 4, QC]
    rt = _rot_matrix()

    in_maps = []
    for c in range(N_CORES):
        b, j = divmod(c, 4)
        xT = np.ascontiguousarray(x[b].T)
        heads = [8 * j + h for h in HEAD_ORDER]
        qcols = np.concatenate([np.arange(h * HD, (h + 1) * HD) for h in heads])
        wq_p = np.ascontiguousarray(Wq[:, qcols])
        wo_p = np.ascontiguousarray(Wo[qcols, :])
        wk_s = np.ascontiguousarray(Wk[:, j * KVCOLS:(j + 1) * KVCOLS])
        wv_s = np.ascontiguousarray(Wv[:, j * KVCOLS:(j + 1) * KVCOLS])
        in_maps.append({
            "xT": xT, "wq": wq_p, "wk": wk_s, "wv": wv_s, "wo": wo_p,
            "cs": cs, "sn": sn, "mk": mk, "rt": rt,
        })
    return in_maps


def kernel(x, cos, sin, Wq, Wk, Wv, Wo, attention_mask):
    x = np.asarray(x, dtype=np.float32)
    cos = np.asarray(cos, dtype=np.float32)
    sin = np.asarray(sin, dtype=np.float32)
    Wq = np.asarray(Wq, dtype=np.float32)
    Wk = np.asarray(Wk, dtype=np.float32)
    Wv = np.asarray(Wv, dtype=np.float32)
    Wo = np.asarray(Wo, dtype=np.float32)
    am = np.asarray(attention_mask).reshape(S, S)
    if np.array_equal(am != 0, np.tril(np.ones((S, S), dtype=bool))):
        causal = True
    elif (am != 0).all():
        causal = False
    else:
        raise NotImplementedError("only causal or all-ones masks supported")

    nc = _get_nc(causal)
    in_maps = _make_in_maps(x, cos, sin, Wq, Wk, Wv, Wo)
    res = run_bass_kernel_spmd(nc, in_maps, core_ids=list(range(N_CORES)))
    out = np.empty((B, S, D), dtype=np.float32)
    for b in range(B):
        acc = res.results[4 * b]["y"].astype(np.float32).copy()
        for j in range(1, 4):
            acc += res.results[4 * b + j]["y"]
        out[b] = acc
    return out
